# revision 1
# baseline (speedup 1.0000x reference)
"""Trainium2 Bass kernel for nn_MultiHeadSliddingWindowAttention.

The reference scatters the 3 sliding-window scores into COLUMNS 0..2 of the
[B,H,N,N] score tensor (faithful-to-source), then softmaxes over all N
columns.  Algebraically the whole attention collapses to, per (b, h, row i):

    out_i = (e0_i*V0 + e1_i*V1 + e2_i*V2 + C) / Z_i
    e_d   = exp(s_d),  s_0 = Q_i.K_{i-1}, s_1 = Q_i.K_i, s_2 = Q_i.K_{i+1}
            (s_d = 0 when the neighbour row does not exist)
    Z_i   = e0 + e1 + e2 + (N-3)
    V0..2 = first three rows of V;  C = sum_{j>=3} V_j

so the [N,N] score tensor never needs to be materialized.  Sharding: 8 cores
= 2 batches x 4 sequence chunks of 512 rows; each core computes Q/K for its
chunk (+1-row halo), the tiny VC4 term, and the full output projection for
its rows.  All activations are kept transposed ([channel, row]) on device so
every matmul contracts over partitions with no on-device transposes.
"""

import os
import numpy as np

B, N, E = 2, 2048, 512
H, DQ = 8, 64
NCHUNK = 4           # sequence chunks per batch
CH = N // NCHUNK     # 512 rows per core
NCORES = 8
NM3 = float(N - 3)   # 2045

last_exec_time_ns = None
_prog = None

# float32r needs a rearranged storage format (walrus checkMatmultFP32r
# rejects plain-fp32 bitcasts), so it stays off; plain fp32 is exact.
F32R = os.environ.get("KERNEL_F32R", "0") == "1"


def _build_program():
    import concourse.bacc as bacc
    import concourse.mybir as mybir
    import concourse.tile as tile

    dt = mybir.dt.float32
    nc = bacc.Bacc(
        "TRN2",
        target_bir_lowering=False,
        debug=False,
        enable_asserts=False,
        num_devices=NCORES,
    )

    def din(name, shape):
        return nc.dram_tensor(name, shape, dt, kind="ExternalInput").ap()

    xt = din("xt", [513, 514])       # x.T halo chunk + ones row (0 at pads)
    xc4 = din("xc4", [513, 32])      # [x0,x1,x2,sum x3:].T cols (m%4) + bmul row
    wqt = din("wqt", [512, 512])     # Wq.T
    wkt = din("wkt", [512, 512])
    wvt = din("wvt", [512, 512])
    wot = din("wot", [512, 512])
    bqc = din("bqc", [512, 1])   # per-channel bias columns (ACT Identity bias)
    boc = din("boc", [512, 1])
    bk = din("bk", [1, 512])
    bv = din("bv", [1, 512])
    hsel = din("hsel", [128, 384])   # head-select matmul weights per (d,t)
    hmask = din("hmask", [32, 512])  # column-block mask for L
    blk = din("blk", [32, 32])       # block-diag Z reduction (ones + 2045 row)
    yt = nc.dram_tensor("yt", [512, 512], dt, kind="ExternalOutput").ap()

    with tile.TileContext(nc) as tc:
        _device_body(tc, mybir, dt, xt, xc4, wqt, wkt, wvt, wot,
                     bqc, boc, bk, bv, hsel, hmask, blk, yt)
    nc.compile()
    return nc


def _device_body(tc, mybir, dt, xt, xc4, wqt, wkt, wvt, wot,
                 bqc, boc, bk, bv, hsel, hmask, blk, yt):
    from contextlib import ExitStack

    nc = tc.nc
    with ExitStack() as ctx:
        const = ctx.enter_context(tc.tile_pool(name="const", bufs=1))
        work = ctx.enter_context(tc.tile_pool(name="work", bufs=4))
        psum = ctx.enter_context(tc.tile_pool(name="psum", bufs=3, space="PSUM"))
        psum2 = ctx.enter_context(tc.tile_pool(name="psum2", bufs=2, space="PSUM"))
        psum_s = ctx.enter_context(tc.tile_pool(name="psums", bufs=1, space="PSUM"))

        def load(tag, src, p, f):
            t = const.tile([p, f], dt, tag=tag)
            nc.sync.dma_start(out=t[:, :], in_=src)
            return t

        xt_sb = [load(f"xt{k}", xt[128 * k:128 * (k + 1), :], 128, 514) for k in range(4)]
        ones = load("ones", xt[512:513, :], 1, 514)
        wq_sb = [load(f"wq{k}", wqt[128 * k:128 * (k + 1), :], 128, 512) for k in range(4)]
        wk_sb = [load(f"wk{k}", wkt[128 * k:128 * (k + 1), :], 128, 512) for k in range(4)]
        wv_sb = [load(f"wv{k}", wvt[128 * k:128 * (k + 1), :], 128, 512) for k in range(4)]
        wo_sb = [load(f"wo{k}", wot[128 * k:128 * (k + 1), :], 128, 512) for k in range(4)]
        xc_sb = [load(f"xc{k}", xc4[128 * k:128 * (k + 1), :], 128, 32) for k in range(4)]
        bmul = load("bmul", xc4[512:513, :], 1, 32)
        bqc_sb = [load(f"bqc{m}", bqc[128 * m:128 * (m + 1), :], 128, 1) for m in range(4)]
        boc_sb = [load(f"boc{m}", boc[128 * m:128 * (m + 1), :], 128, 1) for m in range(4)]
        bk_sb = load("bk", bk[:, :], 1, 512)
        bv_sb = load("bv", bv[:, :], 1, 512)
        hsel_sb = load("hsel", hsel[:, :], 128, 384)
        hmask_sb = load("hmask", hmask[:, :], 32, 512)
        blk_sb = load("blk", blk[:, :], 32, 32)

        ts = lambda i: slice(128 * i, 128 * (i + 1))
        if F32R:
            rr = lambda ap: ap.bitcast(mybir.dt.float32r)
        else:
            rr = lambda ap: ap

        # ---- Q projection: Qt[m] = [128 ch_out, 512 rows] ----
        qt_sb = []
        for m in range(4):
            ps = psum.tile([128, 512], dt, tag="mm")
            for k in range(4):
                nc.tensor.matmul(ps[:, :], rr(wq_sb[k][:, ts(m)]), rr(xt_sb[k][:, 1:513]),
                                 start=(k == 0), stop=(k == 3))
            q = const.tile([128, 512], dt, tag=f"qt{m}")
            nc.scalar.activation(q[:, :], ps[:, :],
                                 mybir.ActivationFunctionType.Identity,
                                 bias=bqc_sb[m][:, 0:1])
            qt_sb.append(q)

        # ---- K projection with halo: Kt[m] = [128 ch_out, 514 rows] ----
        kt_sb = []
        for m in range(4):
            kt = const.tile([128, 514], dt, tag=f"kt{m}")
            ps = psum.tile([128, 512], dt, tag="mm")
            # K keeps bias-as-matmul: the xt ones-row is 0 at pad columns,
            # which zeroes K(pad) exactly (edge rows must see s_d = 0).
            for k in range(4):
                nc.tensor.matmul(ps[:, :], rr(wk_sb[k][:, ts(m)]), rr(xt_sb[k][:, 0:512]),
                                 start=(k == 0), stop=False)
            nc.tensor.matmul(ps[:, :], rr(bk_sb[0:1, ts(m)]), rr(ones[0:1, 0:512]),
                             start=False, stop=True)
            nc.vector.tensor_copy(kt[:, 0:512], ps[:, :])
            ps2 = psum2.tile([128, 2], dt, tag="mm2")
            for k in range(4):
                nc.tensor.matmul(ps2[:, :], wk_sb[k][:, ts(m)], xt_sb[k][:, 512:514],
                                 start=(k == 0), stop=False)
            nc.tensor.matmul(ps2[:, :], bk_sb[0:1, ts(m)], ones[0:1, 512:514],
                             start=False, stop=True)
            nc.vector.tensor_copy(kt[:, 512:514], ps2[:, :])
            kt_sb.append(kt)

        # ---- VC4 (V0,V1,V2,C broadcast to 8 head blocks) + mask -> L ----
        psv = psum_s.tile([32, 512], dt, tag="vc")
        for k in range(4):
            nc.tensor.matmul(psv[:, :], rr(xc_sb[k][:, :]), rr(wv_sb[k][:, :]),
                             start=(k == 0), stop=False)
        nc.tensor.matmul(psv[:, :], rr(bmul[0:1, :]), rr(bv_sb[0:1, :]),
                         start=False, stop=True)
        l_sb = const.tile([32, 512], dt, tag="l")
        nc.vector.tensor_mul(l_sb[:, :], psv[:, :], hmask_sb[:, :])

        # ---- scores S[4h+d, i] = sum_ch Q*K_shift (partition-reduced by hsel) ----
        pss = psum_s.tile([32, 512], dt, tag="s")
        idx = 0
        for d in range(3):
            for t in range(4):
                qk = work.tile([128, 512], dt, tag="qk")
                nc.vector.tensor_mul(qk[:, :], qt_sb[t][:, :], kt_sb[t][:, d:d + 512])
                nc.tensor.matmul(pss[:, :], rr(hsel_sb[:, 32 * idx:32 * (idx + 1)]),
                                 rr(qk[:, :]), start=(idx == 0), stop=(idx == 11))
                idx += 1

        # ---- E = exp(S); Z = blk.T @ E; Ehat = E / Z ----
        e_sb = const.tile([32, 512], dt, tag="e")
        nc.scalar.activation(e_sb[:, :], pss[:, :], mybir.ActivationFunctionType.Exp)
        psz = psum_s.tile([32, 512], dt, tag="z")
        nc.tensor.matmul(psz[:, :], rr(blk_sb[:, :]), rr(e_sb[:, :]),
                         start=True, stop=True)
        r_sb = const.tile([32, 512], dt, tag="r")
        nc.vector.reciprocal(r_sb[:, :], psz[:, :])
        eh_sb = const.tile([32, 512], dt, tag="eh")
        nc.vector.tensor_mul(eh_sb[:, :], e_sb[:, :], r_sb[:, :])

        # ---- outT[t] = L[:, t].T @ Ehat ----
        o_sb = []
        for t in range(4):
            pso = psum.tile([128, 512], dt, tag="mm")
            nc.tensor.matmul(pso[:, :], rr(l_sb[:, ts(t)]), rr(eh_sb[:, :]),
                             start=True, stop=True)
            o = const.tile([128, 512], dt, tag=f"o{t}")
            nc.vector.tensor_copy(o[:, :], pso[:, :])
            o_sb.append(o)

        # ---- output projection: yT[m] = sum_k Wo.T[k, m].T @ outT[k] + bo ----
        for m in range(4):
            psy = psum.tile([128, 512], dt, tag="mm")
            for k in range(4):
                nc.tensor.matmul(psy[:, :], rr(wo_sb[k][:, ts(m)]), rr(o_sb[k][:, :]),
                                 start=(k == 0), stop=(k == 3))
            y = work.tile([128, 512], dt, tag="y")
            nc.scalar.activation(y[:, :], psy[:, :],
                                 mybir.ActivationFunctionType.Identity,
                                 bias=boc_sb[m][:, 0:1])
            nc.sync.dma_start(out=yt[ts(m), :], in_=y[:, :])


def _host_constants():
    hsel = np.zeros((128, 384), np.float32)
    for d in range(3):
        for t in range(4):
            for p in range(128):
                m = 4 * (2 * t + p // 64) + d
                hsel[p, 32 * (4 * d + t) + m] = 1.0
    hmask = np.zeros((32, 512), np.float32)
    for k in range(32):
        hmask[k, (k // 4) * 64:(k // 4 + 1) * 64] = 1.0
    blk = np.zeros((32, 32), np.float32)
    for k in range(32):
        for m in range(32):
            if k // 4 == m // 4:
                blk[k, m] = NM3 if k % 4 == 3 else 1.0
    return hsel, hmask, blk


def kernel(**inputs):
    global _prog, last_exec_time_ns
    from concourse.bass_utils import run_bass_kernel_spmd

    x = np.ascontiguousarray(np.asarray(inputs["x"], dtype=np.float32))
    wqt = np.ascontiguousarray(np.asarray(inputs["Wq"], np.float32).T)
    wkt = np.ascontiguousarray(np.asarray(inputs["Wk"], np.float32).T)
    wvt = np.ascontiguousarray(np.asarray(inputs["Wv"], np.float32).T)
    wot = np.ascontiguousarray(np.asarray(inputs["Wo"], np.float32).T)
    bqc = np.asarray(inputs["bq"], np.float32).reshape(E, 1)
    boc = np.asarray(inputs["bo"], np.float32).reshape(E, 1)
    bk = np.asarray(inputs["bk"], np.float32).reshape(1, E)
    bv = np.asarray(inputs["bv"], np.float32).reshape(1, E)
    hsel, hmask, blk = _host_constants()

    # per-batch xc4: columns cycle [x0, x1, x2, sum_{j>=3} x_j]; last row bmul
    xc4 = np.zeros((B, 513, 32), np.float32)
    bmul = np.array([1.0, 1.0, 1.0, NM3], np.float32)
    for b in range(B):
        cols = np.stack([x[b, 0], x[b, 1], x[b, 2], x[b, 3:].sum(0)], axis=1)
        xc4[b, 0:512, :] = cols[:, np.tile(np.arange(4), 8)]
        xc4[b, 512, :] = bmul[np.tile(np.arange(4), 8)]

    shared = {"wqt": wqt, "wkt": wkt, "wvt": wvt, "wot": wot,
              "bqc": bqc, "boc": boc, "bk": bk, "bv": bv,
              "hsel": hsel, "hmask": hmask, "blk": blk}
    in_maps = []
    for c in range(NCORES):
        b, j = divmod(c, NCHUNK)
        s = j * CH
        xtc = np.zeros((513, 514), np.float32)
        g0 = s - 1
        lo, hi = max(0, g0), min(N, s + CH + 1)
        xtc[0:512, lo - g0:hi - g0] = x[b, lo:hi, :].T
        xtc[512, lo - g0:hi - g0] = 1.0
        in_maps.append({"xt": xtc, "xc4": xc4[b], **shared})

    if _prog is None:
        _prog = _build_program()

    trace = os.environ.get("KERNEL_TRACE", "0") == "1"
    try:
        res = run_bass_kernel_spmd(_prog, in_maps, list(range(NCORES)), trace=trace)
    except ModuleNotFoundError:
        # NTFF profiling hook unavailable in this axon client; run untraced.
        res = run_bass_kernel_spmd(_prog, in_maps, list(range(NCORES)), trace=False)
    last_exec_time_ns = res.exec_time_ns

    y = np.empty((B, N, E), np.float32)
    for c in range(NCORES):
        b, j = divmod(c, NCHUNK)
        y[b, j * CH:(j + 1) * CH, :] = res.results[c]["yt"].T
    return y



# revision 6
# speedup vs baseline: 2.2261x; 2.2261x over previous
"""Trainium2 Bass kernel for nn_MultiHeadSliddingWindowAttention.

The reference scatters the 3 sliding-window scores into COLUMNS 0..2 of the
[B,H,N,N] score tensor (faithful-to-source), then softmaxes over all N
columns.  Algebraically the whole attention collapses to, per (b, h, row i):

    out_i = (e0_i*V0 + e1_i*V1 + e2_i*V2 + C) / Z_i
    e_d   = exp(s_d),  s_0 = Q_i.K_{i-1}, s_1 = Q_i.K_i, s_2 = Q_i.K_{i+1}
            (s_d = 0 when the neighbour row does not exist)
    Z_i   = e0 + e1 + e2 + (N-3)
    V0..2 = first three rows of V;  C = sum_{j>=3} V_j

so the [N,N] score tensor never needs to be materialized.  Sharding: 8 cores
= 2 batches x 4 sequence chunks of 512 rows; each core computes Q/K for its
chunk (+1-row halo) and the full output projection for its rows.

v2: all matmuls in bf16 (fp32 matmul is 4 cycles/row on TRN2, bf16 is 1) and
the output projection is refactored: instead of out = L.T @ Ehat followed by
y = Wo.T @ out (20 full matmuls), precompute G = L @ Wo.T ([32, 512], cheap
because L's row space is only 32) and take y = G.T @ Ehat (8 matmuls).
Activations stay transposed ([channel, row]) so every matmul contracts over
partitions with no on-device transposes.
"""

import os
import numpy as np
import ml_dtypes

B, N, E = 2, 2048, 512
H, DQ = 8, 64
NCHUNK = 4           # sequence chunks per batch
CH = N // NCHUNK     # 512 rows per core
NCORES = 8
NM3 = float(N - 3)   # 2045

last_exec_time_ns = None
_prog = None


def _build_program():
    import concourse.bacc as bacc
    import concourse.mybir as mybir
    import concourse.tile as tile

    bf = mybir.dt.bfloat16
    f32 = mybir.dt.float32
    nc = bacc.Bacc(
        "TRN2",
        target_bir_lowering=False,
        debug=False,
        enable_asserts=False,
        num_devices=NCORES,
    )

    def din(name, shape, dt=bf):
        return nc.dram_tensor(name, shape, dt, kind="ExternalInput").ap()

    xt = din("xt", [513, 514])       # x.T halo chunk + ones row (0 at pads)
    xc4 = din("xc4", [513, 32])      # [x0,x1,x2,sum x3:].T cols (m%4) + bmul row
    wqt = din("wqt", [512, 512])     # Wq.T
    wkt = din("wkt", [512, 512])
    wvt = din("wvt", [512, 512])
    wot = din("wot", [512, 512])
    bqc = din("bqc", [512, 1], f32)  # per-channel bias columns (ACT Identity bias)
    boc = din("boc", [512, 1], f32)
    bk = din("bk", [1, 512])
    bv = din("bv", [1, 512])
    hsel = din("hsel", [128, 384])   # head-select matmul weights per (t,d)
    hmt = din("hmt", [512, 32])      # column mask for L.T (per-channel head select)
    blk = din("blk", [32, 32])       # block-diag Z reduction (ones + ~NM3 row)
    yt = nc.dram_tensor("yt", [512, 512], f32, kind="ExternalOutput").ap()

    with tile.TileContext(nc) as tc:
        _device_body(tc, mybir, bf, f32, xt, xc4, wqt, wkt, wvt, wot,
                     bqc, boc, bk, bv, hsel, hmt, blk, yt)
    nc.compile()
    return nc


def _device_body(tc, mybir, bf, f32, xt, xc4, wqt, wkt, wvt, wot,
                 bqc, boc, bk, bv, hsel, hmt, blk, yt):
    from contextlib import ExitStack

    nc = tc.nc
    with ExitStack() as ctx:
        const = ctx.enter_context(tc.tile_pool(name="const", bufs=1))
        work = ctx.enter_context(tc.tile_pool(name="work", bufs=6))
        psum = ctx.enter_context(tc.tile_pool(name="psum", bufs=3, space="PSUM"))
        psum2 = ctx.enter_context(tc.tile_pool(name="psum2", bufs=2, space="PSUM"))
        psum_s = ctx.enter_context(tc.tile_pool(name="psums", bufs=1, space="PSUM"))

        def load(tag, src, p, q, dt=bf):
            t = const.tile([p, q], dt, tag=tag)
            nc.sync.dma_start(out=t[:, :], in_=src)
            return t

        # DMA order tracks PE consumption order: x first, then Wq, Wk, ...
        xt_sb = [load(f"xt{k}", xt[128 * k:128 * (k + 1), :], 128, 514) for k in range(4)]
        ones = load("ones", xt[512:513, :], 1, 514)
        bqc_sb = [load(f"bqc{m}", bqc[128 * m:128 * (m + 1), :], 128, 1, f32) for m in range(4)]
        wq_sb = [load(f"wq{k}", wqt[128 * k:128 * (k + 1), :], 128, 512) for k in range(4)]
        bk_sb = load("bk", bk[:, :], 1, 512)
        wk_sb = [load(f"wk{k}", wkt[128 * k:128 * (k + 1), :], 128, 512) for k in range(4)]
        hsel_sb = load("hsel", hsel[:, :], 128, 384)
        xc_sb = [load(f"xc{k}", xc4[128 * k:128 * (k + 1), :], 128, 32) for k in range(4)]
        bmul = load("bmul", xc4[512:513, :], 1, 32)
        bv_sb = load("bv", bv[:, :], 1, 512)
        wv_sb = [load(f"wv{k}", wvt[128 * k:128 * (k + 1), :], 128, 512) for k in range(4)]
        hmt_sb = [load(f"hmt{m}", hmt[128 * m:128 * (m + 1), :], 128, 32) for m in range(4)]
        wo_sb = [load(f"wo{k}", wot[128 * k:128 * (k + 1), :], 128, 512) for k in range(4)]
        blk_sb = load("blk", blk[:, :], 32, 32)
        boc_sb = [load(f"boc{m}", boc[128 * m:128 * (m + 1), :], 128, 1, f32) for m in range(4)]

        ts = lambda i: slice(128 * i, 128 * (i + 1))

        # ---- Q projection: Qt[m] = [128 ch_out, 512 rows] (bf16) ----
        qt_sb = []
        for m in range(4):
            ps = psum.tile([128, 512], f32, tag="mm")
            for k in range(4):
                nc.tensor.matmul(ps[:, :], wq_sb[k][:, ts(m)], xt_sb[k][:, 1:513],
                                 start=(k == 0), stop=(k == 3))
            q = const.tile([128, 512], bf, tag=f"qt{m}")
            nc.scalar.activation(q[:, :], ps[:, :],
                                 mybir.ActivationFunctionType.Identity,
                                 bias=bqc_sb[m][:, 0:1])
            qt_sb.append(q)

        # ---- K projection with halo: Kt[m] = [128 ch_out, 514 rows] (bf16) ----
        # K keeps bias-as-matmul: the xt ones-row is 0 at pad columns, which
        # zeroes K(pad) exactly (edge rows must see s_d = 0).
        # Interleave per-m so DVE can form Q*K products while PE still runs K.
        kt_sb = []
        qk_sb = []       # qk tiles in (t, d) order, t-major
        for m in range(4):
            kt = const.tile([128, 514], bf, tag=f"kt{m}")
            ps = psum.tile([128, 512], f32, tag="mm")
            for k in range(4):
                nc.tensor.matmul(ps[:, :], wk_sb[k][:, ts(m)], xt_sb[k][:, 0:512],
                                 start=(k == 0), stop=False)
            nc.tensor.matmul(ps[:, :], bk_sb[0:1, ts(m)], ones[0:1, 0:512],
                             start=False, stop=True)
            nc.vector.tensor_copy(kt[:, 0:512], ps[:, :])
            ps2 = psum2.tile([128, 2], f32, tag="mm2")
            for k in range(4):
                nc.tensor.matmul(ps2[:, :], wk_sb[k][:, ts(m)], xt_sb[k][:, 512:514],
                                 start=(k == 0), stop=False)
            nc.tensor.matmul(ps2[:, :], bk_sb[0:1, ts(m)], ones[0:1, 512:514],
                             start=False, stop=True)
            nc.vector.tensor_copy(kt[:, 512:514], ps2[:, :])
            kt_sb.append(kt)
            for d in range(3):
                qk = work.tile([128, 512], bf, tag=f"qk{m}{d}")
                nc.vector.tensor_mul(qk[:, :], qt_sb[m][:, :], kt[:, d:d + 512])
                qk_sb.append(qk)

        # ---- scores S[4h+d, i] = sum_ch Q*K_shift (partition-reduced by hsel) ----
        # hsel consumes qk tiles t-major (idx = 3t + d) so the last tiles PE
        # needs are the ones DVE finishes last.
        pss = psum_s.tile([32, 512], f32, tag="s")
        for idx in range(12):
            nc.tensor.matmul(pss[:, :], hsel_sb[:, 32 * idx:32 * (idx + 1)],
                             qk_sb[idx][:, :], start=(idx == 0), stop=(idx == 11))

        # ---- E = exp(S) (bf16); Z = blk.T @ E; r = 1/Z; Ehat = E*r ----
        e_sb = const.tile([32, 512], bf, tag="e")
        nc.scalar.activation(e_sb[:, :], pss[:, :], mybir.ActivationFunctionType.Exp)
        psz = psum_s.tile([32, 512], f32, tag="z")
        nc.tensor.matmul(psz[:, :], blk_sb[:, :], e_sb[:, :], start=True, stop=True)
        r_sb = const.tile([32, 512], bf, tag="r")
        with nc.allow_low_precision(reason="1/Z fits bf16; tolerance 2e-2"):
            nc.vector.reciprocal(r_sb[:, :], psz[:, :])
        eh_sb = const.tile([32, 512], bf, tag="eh")
        nc.vector.tensor_mul(eh_sb[:, :], e_sb[:, :], r_sb[:, :])

        # ---- L.T via psvT: [128 ch, 32 (4h+d)] then G = (L.T).T-contract Wo.T ----
        lt_sb = []
        for m in range(4):
            psv = psum2.tile([128, 32], f32, tag="mm2")
            for k in range(4):
                nc.tensor.matmul(psv[:, :], wv_sb[k][:, ts(m)], xc_sb[k][:, :],
                                 start=(k == 0), stop=False)
            nc.tensor.matmul(psv[:, :], bv_sb[0:1, ts(m)], bmul[0:1, :],
                             start=False, stop=True)
            lt = const.tile([128, 32], bf, tag=f"lt{m}")
            nc.vector.tensor_mul(lt[:, :], psv[:, :], hmt_sb[m][:, :])
            lt_sb.append(lt)

        psg = psum_s.tile([32, 512], f32, tag="s")
        for m in range(4):
            nc.tensor.matmul(psg[:, :], lt_sb[m][:, :], wo_sb[m][:, :],
                             start=(m == 0), stop=(m == 3))
        g_sb = const.tile([32, 512], bf, tag="g")
        nc.vector.tensor_copy(g_sb[:, :], psg[:, :])

        # ---- y[m] = G[:, m].T @ Ehat + bo ----
        for m in range(4):
            psy = psum.tile([128, 512], f32, tag="mm")
            nc.tensor.matmul(psy[:, :], g_sb[:, ts(m)], eh_sb[:, :],
                             start=True, stop=True)
            y = work.tile([128, 512], f32, tag="y")
            nc.scalar.activation(y[:, :], psy[:, :],
                                 mybir.ActivationFunctionType.Identity,
                                 bias=boc_sb[m][:, 0:1])
            nc.sync.dma_start(out=yt[ts(m), :], in_=y[:, :])


def _host_constants():
    # hsel consumed t-major: idx = 3t + d
    hsel = np.zeros((128, 384), np.float32)
    for t in range(4):
        for d in range(3):
            for p in range(128):
                m = 4 * (2 * t + p // 64) + d
                hsel[p, 32 * (3 * t + d) + m] = 1.0
    # hmt[ch, 4h+d] = 1 iff channel ch belongs to head h
    hmt = np.zeros((512, 32), np.float32)
    for ch in range(512):
        h = ch // 64
        hmt[ch, 4 * h:4 * h + 4] = 1.0
    blk = np.zeros((32, 32), np.float32)
    for k in range(32):
        for m in range(32):
            if k // 4 == m // 4:
                blk[k, m] = NM3 if k % 4 == 3 else 1.0
    return hsel, hmt, blk


def _bf(a):
    return np.ascontiguousarray(np.asarray(a).astype(ml_dtypes.bfloat16))


def kernel(**inputs):
    global _prog, last_exec_time_ns
    from concourse.bass_utils import run_bass_kernel_spmd

    x = np.ascontiguousarray(np.asarray(inputs["x"], dtype=np.float32))
    wqt = _bf(np.asarray(inputs["Wq"], np.float32).T)
    wkt = _bf(np.asarray(inputs["Wk"], np.float32).T)
    wvt = _bf(np.asarray(inputs["Wv"], np.float32).T)
    wot = _bf(np.asarray(inputs["Wo"], np.float32).T)
    bqc = np.asarray(inputs["bq"], np.float32).reshape(E, 1)
    boc = np.asarray(inputs["bo"], np.float32).reshape(E, 1)
    bk = _bf(np.asarray(inputs["bk"], np.float32).reshape(1, E))
    bv = _bf(np.asarray(inputs["bv"], np.float32).reshape(1, E))
    hsel, hmt, blk = _host_constants()

    # per-batch xc4: columns cycle [x0, x1, x2, sum_{j>=3} x_j]; last row bmul
    xc4 = np.zeros((B, 513, 32), np.float32)
    bmul = np.array([1.0, 1.0, 1.0, NM3], np.float32)
    for b in range(B):
        cols = np.stack([x[b, 0], x[b, 1], x[b, 2], x[b, 3:].sum(0)], axis=1)
        xc4[b, 0:512, :] = cols[:, np.tile(np.arange(4), 8)]
        xc4[b, 512, :] = bmul[np.tile(np.arange(4), 8)]

    shared = {"wqt": wqt, "wkt": wkt, "wvt": wvt, "wot": wot,
              "bqc": bqc, "boc": boc, "bk": bk, "bv": bv,
              "hsel": _bf(hsel), "hmt": _bf(hmt), "blk": _bf(blk)}
    in_maps = []
    for c in range(NCORES):
        b, j = divmod(c, NCHUNK)
        s = j * CH
        xtc = np.zeros((513, 514), np.float32)
        g0 = s - 1
        lo, hi = max(0, g0), min(N, s + CH + 1)
        xtc[0:512, lo - g0:hi - g0] = x[b, lo:hi, :].T
        xtc[512, lo - g0:hi - g0] = 1.0
        in_maps.append({"xt": _bf(xtc), "xc4": _bf(xc4[b]), **shared})

    if _prog is None:
        _prog = _build_program()

    trace = os.environ.get("KERNEL_TRACE", "0") == "1"
    try:
        res = run_bass_kernel_spmd(_prog, in_maps, list(range(NCORES)), trace=trace)
    except ModuleNotFoundError:
        # NTFF profiling hook unavailable in this axon client; run untraced.
        res = run_bass_kernel_spmd(_prog, in_maps, list(range(NCORES)), trace=False)
    last_exec_time_ns = res.exec_time_ns

    y = np.empty((B, N, E), np.float32)
    for c in range(NCORES):
        b, j = divmod(c, NCHUNK)
        y[b, j * CH:(j + 1) * CH, :] = res.results[c]["yt"].T
    return y
